# revision 3
# baseline (speedup 1.0000x reference)
"""MLA forward, 8-core TP redesign (v2).

Two-stage projection replaces the fused A@B weights of v1:
  stage A: feature-sharded A-projection (each core computes 256 of the 2048
    c-features for ALL tokens, reading full hidden^T) + per-token sum-of-squares
    partials; exchanged via 2 AllGathers per batch-half (c in bf16, ss in fp32;
    masked-sum at consumption keeps the program SPMD-uniform).
  stage B: per-head B-projections from the gathered c (2 heads/core) with
    rmsnorm folded in as a per-token output scale (commutes through the matmul).
k_pe is tiny (64 features) and position-dependent: computed + roped on host,
staged like the cos/sin tables, with rows duplicated so the two heads' 64-wide
pe score matmuls row-pack into one PE slot via tile_position (0,0)/(64,0).
Attention keeps scores^T [keys, queries] in PSUM, exp on scalar engine,
denominators via a single ones-matmul over a packed [128,1024] bf16 accumulator.
wo partials stream out in fp16; host sums the 8 partials in fp32.
"""
import sys

sys.path.insert(0, "/opt/trn_rl_repo")

import numpy as np
import ml_dtypes

import concourse.mybir as mybir
from concourse import bacc
from concourse.tile import TileContext
from concourse.bass_utils import run_bass_kernel_spmd

BF16 = ml_dtypes.bfloat16
F32 = mybir.dt.float32
F16 = mybir.dt.float16
BF = mybir.dt.bfloat16

B, S, H = 2, 2048, 2048
NH = 16
Q_LORA, KV_LORA = 1536, 512
D_NOPE, D_ROPE, D_V = 128, 64, 128
D_QK = D_NOPE + D_ROPE
SCALE = 1.0 / float(np.sqrt(D_QK))
EPS = 1e-6

N_CORES = 8
HPC = 2                      # heads per core
TOK = B * S                  # 4096
HTOK = S                     # tokens per half (= batch)
CF = Q_LORA + KV_LORA        # 2048 c-features
KC = H // 128                # 16 contraction chunks over hidden
CC = CF // 128               # 16 c-feature chunks (12 q + 4 kv)
QCH = Q_LORA // 128          # 12


def _host_tables():
    inv = 1.0 / (10000.0 ** (np.arange(0, D_ROPE, 2, dtype=np.float32) / D_ROPE))
    t = np.arange(S, dtype=np.float32)
    f = np.outer(t, inv)                       # (S, 32)
    cosT = np.tile(np.cos(f).T, (1, B))        # (32, TOK), tokens b-major
    sinT = np.tile(np.sin(f).T, (1, B))
    csq1 = np.concatenate([cosT, cosT, sinT, sinT], axis=0)   # (128, TOK)
    csq2 = np.concatenate([sinT, sinT, cosT, cosT], axis=0)
    return (np.ascontiguousarray(csq1).astype(BF16),
            np.ascontiguousarray(csq2).astype(BF16), cosT, sinT)


def _host_prep(hidden_states, wq_a, q_norm_w, wq_b, wkv_a, kv_norm_w, wkv_b, wo):
    hid = np.ascontiguousarray(
        np.asarray(hidden_states, dtype=np.float32).reshape(TOK, H))
    hT_bf = np.ascontiguousarray(hid.T).astype(BF16)           # (H, TOK)

    wq_a = np.asarray(wq_a, dtype=np.float32)
    wkv_a = np.asarray(wkv_a, dtype=np.float32)
    wa = np.concatenate([wq_a, wkv_a[:KV_LORA]], axis=0)       # (CF, H)
    waT = np.ascontiguousarray(wa.T)                           # (H, CF)

    csq1, csq2, cosT, sinT = _host_tables()

    # host k_pe: project + rope (64 features only)
    kpe_raw = hid @ wkv_a[KV_LORA:].T                          # (TOK, 64)
    e = kpe_raw[:, 0::2].T                                     # (32, TOK)
    o = kpe_raw[:, 1::2].T
    re = e * cosT - o * sinT
    im = e * sinT + o * cosT
    kpe64 = np.concatenate([re, im], axis=0)                   # (64, TOK)
    kpe_dup = np.ascontiguousarray(
        np.concatenate([kpe64, kpe64], axis=0)).astype(BF16)   # (128, TOK)

    wq_b_f = (np.asarray(wq_b) * np.asarray(q_norm_w)[None, :]).astype(np.float32)
    wkv_b_f = (np.asarray(wkv_b) * np.asarray(kv_norm_w)[None, :]).astype(np.float32)
    wo = np.asarray(wo, dtype=np.float32)

    masks = np.zeros((8, 2), dtype=np.float32)
    masks[:QCH // 2, 0] = 1.0      # cores 0-5 hold q chunks
    masks[QCH // 2:, 1] = 1.0      # cores 6-7 hold kv chunks

    def chunk_cols(Wstack):
        # (R, F) -> (128, F//128 * R):  block k = Wstack[:, 128k:128k+128].T
        R, F = Wstack.shape
        blocks = [np.ascontiguousarray(Wstack[:, 128 * k:128 * (k + 1)].T)
                  for k in range(F // 128)]
        return np.ascontiguousarray(np.concatenate(blocks, axis=1)).astype(BF16)

    in_maps = []
    for c in range(N_CORES):
        h0, h1 = 2 * c, 2 * c + 1
        Wq0 = wq_b_f[h0 * D_QK:(h0 + 1) * D_QK]                # (192, 1536)
        Wq1 = wq_b_f[h1 * D_QK:(h1 + 1) * D_QK]
        E0, O0 = Wq0[D_NOPE::2], Wq0[D_NOPE + 1::2]            # (32, 1536)
        E1, O1 = Wq1[D_NOPE::2], Wq1[D_NOPE + 1::2]
        Wq_stack = np.concatenate(
            [Wq0[:D_NOPE], Wq1[:D_NOPE], E0, E1, O0, O1], axis=0)  # (384, 1536)
        kv0 = wkv_b_f[h0 * (D_NOPE + D_V):(h0 + 1) * (D_NOPE + D_V)]  # (256, 512)
        kv1 = wkv_b_f[h1 * (D_NOPE + D_V):(h1 + 1) * (D_NOPE + D_V)]
        Wkn_stack = np.concatenate([kv0[:D_NOPE], kv1[:D_NOPE]], axis=0)
        Wv_stack = np.concatenate([kv0[D_NOPE:], kv1[D_NOPE:]], axis=0)

        waT_s = np.ascontiguousarray(waT[:, 256 * c:256 * (c + 1)])  # (H, 256)
        waT_st = np.ascontiguousarray(np.concatenate(
            [waT_s[128 * k:128 * (k + 1), :] for k in range(KC)],
            axis=1)).astype(BF16)                               # (128, 16*256)

        wo_h = wo[:, c * HPC * D_V:(c + 1) * HPC * D_V]         # (H, 256)
        woR = np.ascontiguousarray(wo_h.T).astype(BF16)         # (256, H)

        in_maps.append({
            "hT": hT_bf,
            "waT_st": waT_st,
            "WqbT_st": chunk_cols(Wq_stack),    # (128, 12*384)
            "WknT_st": chunk_cols(Wkn_stack),   # (128, 4*256)
            "WvT_st": chunk_cols(Wv_stack),     # (128, 4*256)
            "woR": woR,
            "kpe_dup": kpe_dup,
            "csq1": csq1, "csq2": csq2,
            "masks": masks,
        })
    return in_maps


def _build_program():
    nc = bacc.Bacc()

    hT = nc.dram_tensor("hT", [H, TOK], BF, kind="ExternalInput")
    waT_st = nc.dram_tensor("waT_st", [128, KC * 256], BF, kind="ExternalInput")
    WqbT_st = nc.dram_tensor("WqbT_st", [128, QCH * 384], BF, kind="ExternalInput")
    WknT_st = nc.dram_tensor("WknT_st", [128, 4 * 256], BF, kind="ExternalInput")
    WvT_st = nc.dram_tensor("WvT_st", [128, 4 * 256], BF, kind="ExternalInput")
    woR = nc.dram_tensor("woR", [HPC * D_V, H], BF, kind="ExternalInput")
    kpe_d = nc.dram_tensor("kpe_dup", [128, TOK], BF, kind="ExternalInput")
    csq1d = nc.dram_tensor("csq1", [128, TOK], BF, kind="ExternalInput")
    csq2d = nc.dram_tensor("csq2", [128, TOK], BF, kind="ExternalInput")
    masksd = nc.dram_tensor("masks", [8, 2], F32, kind="ExternalInput")
    out = nc.dram_tensor("out", [TOK, H], F16, kind="ExternalOutput")

    AF = mybir.ActivationFunctionType
    OP = mybir.AluOpType

    with TileContext(nc) as tc:
        with tc.tile_pool(name="const", bufs=1) as constp, \
             tc.tile_pool(name="wts", bufs=1) as wts, \
             tc.tile_pool(name="acts", bufs=2) as acts, \
             tc.tile_pool(name="dram", bufs=1, space="DRAM") as dram, \
             tc.tile_pool(name="dramsh", bufs=1, space="DRAM") as dramsh:

            ones_col = constp.tile([128, 1], F32, name="ones_col")
            ones_row = constp.tile([1, 128], F32, name="ones_row")
            ones_col_bf = constp.tile([128, 1], BF, name="ones_col_bf")
            nc.vector.memset(ones_col[:], 1.0)
            nc.vector.memset(ones_row[:], 1.0)
            nc.vector.memset(ones_col_bf[:], 1.0)
            eps_col = constp.tile([128, 1], F32, name="eps_col")
            nc.vector.memset(eps_col[:], EPS)
            masks_t = constp.tile([8, 2], F32, name="masks_t")
            nc.gpsimd.dma_start(masks_t[:], masksd[:])

            wa_t = wts.tile([128, KC * 256], BF, name="wa_t")
            nc.gpsimd.dma_start(wa_t[:], waT_st[:])
            wqb_t = wts.tile([128, QCH * 384], BF, name="wqb_t")
            nc.gpsimd.dma_start(wqb_t[:], WqbT_st[:])
            wkn_t = wts.tile([128, 4 * 256], BF, name="wkn_t")
            nc.gpsimd.dma_start(wkn_t[:], WknT_st[:])
            wv_t = wts.tile([128, 4 * 256], BF, name="wv_t")
            nc.gpsimd.dma_start(wv_t[:], WvT_st[:])
            wo_t = []
            for i in range(2):
                t = wts.tile([128, H], BF, tag=f"wot{i}", name=f"wot{i}")
                nc.gpsimd.dma_start(t[:], woR[i * 128:(i + 1) * 128, :])
                wo_t.append(t)
            kpe_t = wts.tile([128, TOK], BF, name="kpe_t")
            nc.gpsimd.dma_start(kpe_t[:], kpe_d[:])
            csq1_t = wts.tile([128, TOK], BF, name="csq1_t")
            csq2_t = wts.tile([128, TOK], BF, name="csq2_t")
            nc.gpsimd.dma_start(csq1_t[:], csq1d[:])
            nc.gpsimd.dma_start(csq2_t[:], csq2d[:])

            ccA_in = [dram.tile([256, HTOK], BF, tag=f"ccAi{h}", name=f"ccAi{h}")
                      for h in range(2)]
            ccA_out = [dramsh.tile([CF, HTOK], BF, tag=f"ccAo{h}",
                                   name=f"ccAo{h}", addr_space="Shared")
                       for h in range(2)]
            ccS_in = [dram.tile([1, HTOK], F32, tag=f"ccSi{h}", name=f"ccSi{h}")
                      for h in range(2)]
            ccS_out = [dram.tile([8, HTOK], F32, tag=f"ccSo{h}", name=f"ccSo{h}")
                       for h in range(2)]

            # ---------------- stage A ------------------------------------
            with tc.tile_pool(name="pA", bufs=2, space="PSUM") as pA, \
                 tc.tile_pool(name="pAss", bufs=2, space="PSUM") as pAss, \
                 tc.tile_pool(name="htp", bufs=1) as htp, \
                 tc.tile_pool(name="aev", bufs=4) as aev:
                for h in range(2):
                    for r in range(1):
                        ht = []
                        for k in range(KC):
                            t = htp.tile([128, 2048], BF, tag=f"ht{k}",
                                         name=f"ht{k}")
                            nc.sync.dma_start(
                                t[:], hT[128 * k:128 * (k + 1),
                                         2048 * h:2048 * (h + 1)])
                            ht.append(t)
                        for sb2 in range(4):
                            lsl = slice(sb2 * 512, (sb2 + 1) * 512)
                            tloc = sb2 * 512
                            ps_ss = pAss.tile([1, 512], F32, tag="ss",
                                              name="ps_ss")
                            for j in range(2):
                                ps_c = pA.tile([128, 512], F32, tag="ps_c",
                                               name="ps_c")
                                for k in range(KC):
                                    nc.tensor.matmul(
                                        ps_c[:],
                                        lhsT=wa_t[:, 256 * k + 128 * j:
                                                  256 * k + 128 * (j + 1)],
                                        rhs=ht[k][:, lsl],
                                        start=(k == 0), stop=(k == KC - 1))
                                c_sb = aev.tile([128, 512], BF, tag="c_sb",
                                                name="c_sb")
                                nc.vector.tensor_copy(c_sb[:], ps_c[:])
                                nc.sync.dma_start(
                                    ccA_in[h][128 * j:128 * (j + 1),
                                              tloc:tloc + 512], c_sb[:])
                                sq_sb = aev.tile([128, 512], F32, tag="sq",
                                                 name="sq_sb")
                                nc.scalar.activation(sq_sb[:], ps_c[:],
                                                     AF.Square)
                                nc.tensor.matmul(ps_ss[:], lhsT=ones_col[:],
                                                 rhs=sq_sb[:],
                                                 start=(j == 0), stop=(j == 1))
                            ss_sb = aev.tile([1, 512], F32, tag="ss_sb",
                                             name="ss_sb")
                            nc.vector.tensor_copy(ss_sb[:], ps_ss[:])
                            nc.sync.dma_start(
                                ccS_in[h][0, tloc:tloc + 512], ss_sb[:])
                    nc.gpsimd.collective_compute(
                        "AllGather", OP.bypass,
                        replica_groups=[list(range(N_CORES))],
                        ins=[ccS_in[h].opt()], outs=[ccS_out[h].opt()])
                    nc.gpsimd.collective_compute(
                        "AllGather", OP.bypass,
                        replica_groups=[list(range(N_CORES))],
                        ins=[ccA_in[h].opt()], outs=[ccA_out[h].opt()])

            # ---------------- inv + stage B ------------------------------
            qn = {}
            kn = {}
            qpe = {}
            vnat = {}
            for h in range(2):
                with tc.tile_pool(name="pB", bufs=3, space="PSUM") as pB, \
                     tc.tile_pool(name="pinv", bufs=2, space="PSUM") as pinv, \
                     tc.tile_pool(name="pcols", bufs=1, space="PSUM") as pcols, \
                     tc.tile_pool(name="cp", bufs=1) as cp, \
                     tc.tile_pool(name="invp", bufs=1) as invp, \
                     tc.tile_pool(name="bev", bufs=2) as bev:

                    # --- inv_rms rows + broadcast ---
                    ssg = invp.tile([8, HTOK], F32, tag="ssg", name="ssg")
                    nc.sync.dma_start(ssg[:], ccS_out[h][:])
                    iqbc = invp.tile([128, HTOK], F32, tag="iqbc", name="iqbc")
                    ikbc = invp.tile([128, HTOK], F32, tag="ikbc", name="ikbc")
                    for n in range(4):
                        nsl = slice(512 * n, 512 * (n + 1))
                        for which, scale, dst in ((0, 1.0 / Q_LORA, iqbc),
                                                  (1, 1.0 / KV_LORA, ikbc)):
                            ps_sum = pinv.tile([1, 512], F32, tag="pssum",
                                               name="ps_sum")
                            nc.tensor.matmul(
                                ps_sum[:],
                                lhsT=masks_t[:, which:which + 1],
                                rhs=ssg[:, nsl], start=True, stop=True)
                            srow = bev.tile([1, 512], F32, tag="srow",
                                            name="srow")
                            nc.scalar.activation(srow[:], ps_sum[:], AF.Sqrt,
                                                 bias=eps_col[0:1, :],
                                                 scale=scale)
                            irow = bev.tile([1, 512], F32, tag="irow",
                                            name="irow")
                            nc.vector.reciprocal(irow[:], srow[:])
                            ps_bc = pinv.tile([128, 512], F32, tag="psbc",
                                              name="ps_bc")
                            nc.tensor.matmul(ps_bc[:], lhsT=ones_row[:],
                                             rhs=irow[:],
                                             start=True, stop=True)
                            nc.vector.tensor_copy(dst[:, nsl], ps_bc[:])
                    # per-token inv_kv columns for the v eviction scale
                    ps_cls = pcols.tile([128, 16], F32, tag="pcols",
                                        name="ps_cols")
                    for tm in range(16):
                        nc.tensor.matmul(
                            ps_cls[:, tm:tm + 1],
                            lhsT=ssg[:, 128 * tm:128 * (tm + 1)],
                            rhs=masks_t[:, 1:2], start=True, stop=True)
                    rms_cols = bev.tile([128, 16], F32, tag="rmsc",
                                        name="rms_cols")
                    nc.scalar.activation(rms_cols[:], ps_cls[:], AF.Sqrt,
                                         bias=eps_col[:], scale=1.0 / KV_LORA)
                    ivk_cols = bev.tile([128, 16], F32, tag="ivkc",
                                        name="ivk_cols")
                    nc.vector.reciprocal(ivk_cols[:], rms_cols[:])

                    # --- persistent activation tiles for this half ---
                    qn[h] = [acts.tile([128, HTOK], BF, tag=f"qn{i}",
                                       name=f"qn{i}_{h}") for i in range(2)]
                    kn[h] = [acts.tile([128, HTOK], BF, tag=f"kn{i}",
                                       name=f"kn{i}_{h}") for i in range(2)]
                    qpe[h] = acts.tile([128, HTOK], BF, tag="qpe",
                                       name=f"qpe_{h}")
                    vnat[h] = [acts.tile([128, HPC * D_V], BF, tag=f"v{tm}",
                                         name=f"v{tm}_{h}") for tm in range(16)]

                    # --- stage B ---
                    for r in range(2):
                        cbl = []
                        for k in range(CC):
                            t = cp.tile([128, 1024], BF, tag=f"c{k}",
                                        name=f"c{k}")
                            nc.sync.dma_start(
                                t[:], ccA_out[h][128 * k:128 * (k + 1),
                                                 1024 * r:1024 * (r + 1)])
                            cbl.append(t)
                        for sb2 in range(2):
                            lsl = slice(sb2 * 512, (sb2 + 1) * 512)
                            tloc = 1024 * r + sb2 * 512
                            tsl = slice(tloc, tloc + 512)
                            tgl = slice(2048 * h + tloc, 2048 * h + tloc + 512)
                            bq = iqbc[:, tsl]
                            bkv = ikbc[:, tsl]
                            # q chunks: qn0, qn1, pe-stack
                            for n in range(3):
                                ps_q = pB.tile([128, 512], F32, tag="psb",
                                               name="ps_q")
                                for k in range(QCH):
                                    nc.tensor.matmul(
                                        ps_q[:],
                                        lhsT=wqb_t[:, 384 * k + 128 * n:
                                                   384 * k + 128 * (n + 1)],
                                        rhs=cbl[k][:, lsl],
                                        start=(k == 0), stop=(k == QCH - 1))
                                if n < 2:
                                    nc.vector.tensor_mul(
                                        qn[h][n][:, tsl], ps_q[:], bq)
                                else:
                                    tq = bev.tile([128, 512], F32, tag="tq",
                                                  name="tq")
                                    nc.vector.tensor_mul(tq[:], ps_q[:], bq)
                                    # split-m rope: every DVE input pair shares
                                    # a base partition (walrus I-803)
                                    m1a = bev.tile([64, 512], F32, tag="m1a",
                                                   name="m1a")
                                    m1b = bev.tile([64, 512], F32, tag="m1b",
                                                   name="m1b")
                                    m2a = bev.tile([64, 512], F32, tag="m2a",
                                                   name="m2a")
                                    m2b = bev.tile([64, 512], F32, tag="m2b",
                                                   name="m2b")
                                    nc.vector.tensor_mul(
                                        m1a[:], tq[0:64, :],
                                        csq1_t[0:64, tgl])
                                    nc.vector.tensor_mul(
                                        m1b[:], tq[64:128, :],
                                        csq1_t[64:128, tgl])
                                    nc.vector.tensor_mul(
                                        m2a[:], tq[0:64, :],
                                        csq2_t[0:64, tgl])
                                    nc.vector.tensor_mul(
                                        m2b[:], tq[64:128, :],
                                        csq2_t[64:128, tgl])
                                    qp = qpe[h]
                                    nc.vector.tensor_sub(
                                        qp[0:32, tsl], m1a[0:32, :],
                                        m1b[0:32, :])
                                    nc.vector.tensor_add(
                                        qp[32:64, tsl], m2a[0:32, :],
                                        m2b[0:32, :])
                                    nc.vector.tensor_sub(
                                        qp[64:96, tsl], m1a[32:64, :],
                                        m1b[32:64, :])
                                    nc.vector.tensor_add(
                                        qp[96:128, tsl], m2a[32:64, :],
                                        m2b[32:64, :])
                            # k_nope chunks
                            for n in range(2):
                                ps_k = pB.tile([128, 512], F32, tag="psb",
                                               name="ps_k")
                                for k in range(4):
                                    nc.tensor.matmul(
                                        ps_k[:],
                                        lhsT=wkn_t[:, 256 * k + 128 * n:
                                                   256 * k + 128 * (n + 1)],
                                        rhs=cbl[QCH + k][:, lsl],
                                        start=(k == 0), stop=(k == 3))
                                nc.vector.tensor_mul(kn[h][n][:, tsl],
                                                     ps_k[:], bkv)
                            # v (token-major), 2 tiles of [128, 2*256]
                            for vt in range(2):
                                ps_v = pB.tile([128, 512], F32, tag="psb",
                                               name="ps_v")
                                for half2 in range(2):
                                    t0 = sb2 * 512 + (2 * vt + half2) * 128
                                    for k in range(4):
                                        nc.tensor.matmul(
                                            ps_v[:, 256 * half2:
                                                 256 * (half2 + 1)],
                                            lhsT=cbl[QCH + k][:, t0:t0 + 128],
                                            rhs=wv_t[:, 256 * k:256 * (k + 1)],
                                            start=(k == 0), stop=(k == 3))
                                for half2 in range(2):
                                    tm = 8 * r + 4 * sb2 + 2 * vt + half2
                                    nc.scalar.activation(
                                        vnat[h][tm][:],
                                        ps_v[:, 256 * half2:256 * (half2 + 1)],
                                        AF.Copy,
                                        scale=ivk_cols[:, tm:tm + 1])

                # ------------- attention + wo for this half --------------
                with tc.tile_pool(name="sps", bufs=2, space="PSUM") as sps, \
                     tc.tile_pool(name="ops", bufs=1, space="PSUM") as ops, \
                     tc.tile_pool(name="wps", bufs=2, space="PSUM") as wps, \
                     tc.tile_pool(name="esb", bufs=3) as esb, \
                     tc.tile_pool(name="asb", bufs=1) as asb, \
                     tc.tile_pool(name="otp", bufs=1) as otp, \
                     tc.tile_pool(name="osb", bufs=2) as osb:
                    outT = [otp.tile([128, HTOK], BF, tag=f"outT{i}",
                                     name=f"outT{i}_{h}") for i in range(2)]
                    for qb in range(4):
                        qsl = slice(qb * 512, (qb + 1) * 512)
                        ps_o = [ops.tile([128, 512], F32, tag=f"o{i}",
                                         name=f"ps_o{i}") for i in range(2)]
                        acc = [asb.tile([128, 1024], BF, tag=f"acc{i}",
                                        name=f"acc{i}") for i in range(2)]
                        for kcc in range(8):
                            for i in range(2):
                                ps_s = sps.tile([128, 1024], F32, tag="s",
                                                name=f"ps_s{i}")
                                for hf in range(2):
                                    kc = 2 * kcc + hf
                                    ksl = slice(kc * 128, (kc + 1) * 128)
                                    kgl = slice(2048 * h + kc * 128,
                                                2048 * h + (kc + 1) * 128)
                                    csl = slice(512 * hf, 512 * (hf + 1))
                                    nc.tensor.matmul(ps_s[:, csl],
                                                     lhsT=kn[h][i][:, ksl],
                                                     rhs=qn[h][i][:, qsl],
                                                     start=True, stop=False)
                                    nc.tensor.matmul(
                                        ps_s[:, csl],
                                        lhsT=kpe_t[64 * i:64 * (i + 1), kgl],
                                        rhs=qpe[h][64 * i:64 * (i + 1), qsl],
                                        start=False, stop=True)
                                ex = esb.tile([128, 1024], BF, tag="exp",
                                              name=f"exp{i}")
                                nc.scalar.activation(ex[:], ps_s[:], AF.Exp,
                                                     scale=SCALE)
                                if kcc == 0:
                                    nc.vector.tensor_copy(acc[i][:], ex[:])
                                else:
                                    nc.vector.tensor_add(acc[i][:], acc[i][:],
                                                         ex[:])
                                for hf in range(2):
                                    kc = 2 * kcc + hf
                                    nc.tensor.matmul(
                                        ps_o[i][:],
                                        lhsT=vnat[h][kc][:,
                                                         128 * i:128 * (i + 1)],
                                        rhs=ex[:, 512 * hf:512 * (hf + 1)],
                                        start=(kc == 0), stop=(kc == 15))
                        for i in range(2):
                            accf = esb.tile([128, 512], BF, tag="accf",
                                            name="accf")
                            nc.vector.tensor_add(accf[:], acc[i][:, 0:512],
                                                 acc[i][:, 512:1024])
                            ps_db = sps.tile([128, 1024], F32, tag="s",
                                             name="ps_db")
                            nc.tensor.matmul(ps_db[0:1, 0:512],
                                             lhsT=ones_col_bf[:],
                                             rhs=accf[:],
                                             start=True, stop=True)
                            rec = esb.tile([1, 512], F32, tag="rec",
                                           name="rec")
                            nc.vector.reciprocal(rec[:], ps_db[0:1, 0:512])
                            nc.tensor.matmul(ps_db[:, 512:1024],
                                             lhsT=ones_row[:], rhs=rec[:],
                                             start=True, stop=True)
                            bc_sb = esb.tile([128, 512], F32, tag="bcs",
                                             name="bc_sb")
                            nc.vector.tensor_copy(bc_sb[:], ps_db[:, 512:1024])
                            nc.vector.tensor_mul(outT[i][:, qsl],
                                                 ps_o[i][:], bc_sb[:])
                    # wo for this batch
                    for tmb in range(16):
                        osl = slice(tmb * 128, (tmb + 1) * 128)
                        o_sb = osb.tile([128, H], F16, tag="osb", name="o_sb")
                        for hn in range(4):
                            nsl = slice(hn * 512, (hn + 1) * 512)
                            ps_w = wps.tile([128, 512], F32, tag="w",
                                            name="ps_w")
                            nc.tensor.matmul(ps_w[:], lhsT=outT[0][:, osl],
                                             rhs=wo_t[0][:, nsl],
                                             start=True, stop=False)
                            nc.tensor.matmul(ps_w[:], lhsT=outT[1][:, osl],
                                             rhs=wo_t[1][:, nsl],
                                             start=False, stop=True)
                            if hn % 2 == 0:
                                nc.vector.tensor_copy(o_sb[:, nsl], ps_w[:])
                            else:
                                nc.scalar.activation(o_sb[:, nsl], ps_w[:],
                                                     AF.Copy)
                        trow = 2048 * h + 128 * tmb
                        nc.sync.dma_start(out[trow:trow + 128, :], o_sb[:])

    nc.compile()
    return nc


_PROGRAM = None


def _get_program():
    global _PROGRAM
    if _PROGRAM is None:
        _PROGRAM = _build_program()
    return _PROGRAM


def kernel(hidden_states, wq_a, q_norm_w, wq_b, wkv_a, kv_norm_w, wkv_b, wo):
    nc = _get_program()
    in_maps = _host_prep(hidden_states, wq_a, q_norm_w, wq_b,
                         wkv_a, kv_norm_w, wkv_b, wo)
    res = run_bass_kernel_spmd(nc, in_maps, list(range(N_CORES)))
    total = np.zeros((TOK, H), dtype=np.float32)
    for r in res.results:
        total += r["out"].astype(np.float32)
    return total.reshape(B, S, H)
